# revision 7
# baseline (speedup 1.0000x reference)
"""Bahdanau attention scores kernel for Trainium2 (8 NeuronCores).

Math (per batch row b):
    energy[s, :] = tanh(hidden[b] @ W_h + enc[s, b, :] @ W_e + bias)
    scores[s]    = energy[s, :] . v
    out[b, :]    = softmax(scores)

Strategy:
  - Data-parallel: batch (32) sharded 4-per-core across 8 cores; weights
    replicated. No cross-core communication (softmax is per-row over seq).
  - All heavy tensors are marshalled on the HOST into device-friendly
    layouts: enc is pre-transposed to [b, e, s] fp16 so every DMA is a
    natural-layout copy (no SWDGE casts, no DMA-xbar transposes), W_e/W_h
    are pre-cast fp16, hidden is pre-transposed [d, b] fp16.
  - PE does only matmuls: 64 tiny hwh matmuls (W_h^T @ hidden^T, slotted
    mid-stream once W_h lands) and the 1024 main matmuls
    energy^T[d, s] += W_e[e, d]^T @ enc^T[e, s] (fp16, 512-col moving).
  - ACT applies tanh with the per-(d, b) bias hWh^T + b, writing fp16.
  - The v-dot moves OFF the PE: DVE runs a fused multiply-accumulate
    chain acc = (th[dc] * v[dc]) + acc over the 8 d-chunks
    (scalar_tensor_tensor), then GpSimd partition_all_reduce sums the
    128 partitions, leaving scores replicated on all partitions.
  - Softmax runs on the replicated [128, 2048] rows (every partition
    computes the same thing, only partition 0 is written out): DVE
    -max via negated reduce, ACT exp with accumulated sum, DVE
    reciprocal + scale, then a contiguous 8 KB DMA of row 0.
"""

import sys

for _p in ("/opt/trn_rl_repo", "/root/.axon_site/_ro/trn_rl_repo"):
    if _p not in sys.path:
        sys.path.append(_p)

from contextlib import ExitStack

import numpy as np

import concourse.bass as bass
import concourse.tile as tile
from concourse import mybir
from concourse.bass_utils import run_bass_kernel_spmd

P = 128
S, B, E, D = 2048, 32, 1024, 1024  # seq, batch, 2*enc_hs, dec_hs
NCORES = 8
BL = B // NCORES  # batches per core
ST = 512  # seq cols per tile
NST = S // ST
EC = E // P  # 8 e-chunks
DC = D // P  # 8 d-chunks

f32 = mybir.dt.float32
f16 = mybir.dt.float16


def _split_multiwaits(nc):
    """This container's walrus rejects >1 semaphore wait per instruction
    ("Too many sync wait commands"); Tile attaches several to its final
    drain. Move extra waits onto dedicated NoOps just before the carrying
    instruction (same engine, program order => identical blocking)."""
    for fn in nc.m.functions:
        for bb in fn.blocks:
            out = []
            changed = False
            for inst in bb.instructions:
                si = inst.sync_info
                waits = list(si.on_wait) if si is not None and si.on_wait else []
                limit = 0 if isinstance(inst, mybir.InstDrain) else 1
                if len(waits) > limit:
                    for w in waits[limit:]:
                        out.append(
                            mybir.InstNoOp(
                                name=nc.get_next_instruction_name(),
                                opcode="NoOp",
                                engine=inst.engine,
                                sync_info=mybir.SyncInfo(on_wait=[w], on_update=[]),
                                text_hint="waitfix",
                                bass_nofuse=True,
                            )
                        )
                    si.on_wait = waits[:limit]
                    changed = True
                out.append(inst)
            if changed:
                bb.instructions.clear()
                for inst in out:
                    bb.instructions.append(inst)


def _build():
    nc = bass.Bass()
    enc = nc.declare_dram_parameter("enc16", [BL, E, S], f16, isOutput=False)
    we = nc.declare_dram_parameter("we16", [E, D], f16, isOutput=False)
    wh = nc.declare_dram_parameter("wh16", [D, D], f16, isOutput=False)
    hid = nc.declare_dram_parameter("hidT16", [D, BL], f16, isOutput=False)
    bvec = nc.declare_dram_parameter("bT32", [D], f32, isOutput=False)
    vvec = nc.declare_dram_parameter("v32", [D], f32, isOutput=False)
    out = nc.declare_dram_parameter("out", [BL, S], f32, isOutput=True)

    with tile.TileContext(nc) as tc, ExitStack() as ctx:
        consts = ctx.enter_context(tc.tile_pool(name="consts", bufs=1))
        encp = ctx.enter_context(tc.tile_pool(name="encp", bufs=3))
        thp = ctx.enter_context(tc.tile_pool(name="thp", bufs=2))
        accp = ctx.enter_context(tc.tile_pool(name="accp", bufs=2))
        acc16p = ctx.enter_context(tc.tile_pool(name="acc16p", bufs=2))
        scorep = ctx.enter_context(tc.tile_pool(name="scorep", bufs=2))
        probp = ctx.enter_context(tc.tile_pool(name="probp", bufs=2))
        smallp = ctx.enter_context(tc.tile_pool(name="smallp", bufs=2))
        psumE = ctx.enter_context(tc.tile_pool(name="psumE", bufs=6, space="PSUM"))
        psumH = ctx.enter_context(tc.tile_pool(name="psumH", bufs=1, space="PSUM"))
        psumS = ctx.enter_context(tc.tile_pool(name="psumS", bufs=1, space="PSUM"))

        weT = we.rearrange("(ec p) d -> p ec d", p=P)
        whT = wh.rearrange("(hc p) d -> p hc d", p=P)

        # ---- DMA issue order ------------------------------------------
        # sync queue: We dc0-slices (tiny, unblock the first psum group),
        # then full We chunks, W_h chunks, and the small tensors.
        # gpsimd queue: enc tiles (t0 split per-ec for fast start).
        We0 = consts.tile([P, EC, P], f16)
        We16 = consts.tile([P, EC, D], f16)
        Wh16 = consts.tile([P, DC, D], f16)
        hidT = consts.tile([P, DC, BL], f16)
        bT32 = consts.tile([P, DC], f32)
        v32 = consts.tile([P, DC], f32)
        ones16 = consts.tile([P, P], f16)
        nc.vector.memset(ones16[:], 1.0)

        encTs = {}
        tiles = [(b, st) for b in range(BL) for st in range(NST)]

        def load_enc_tile(ti, split=False):
            b, st = tiles[ti]
            t = encp.tile([P, EC, ST], f16)
            src = enc[b].rearrange("(ec p) s -> p ec s", p=P)[
                :, :, st * ST : (st + 1) * ST
            ]
            if split:
                for ec in range(EC):
                    nc.gpsimd.dma_start(out=t[:, ec, :], in_=src[:, ec, :])
            else:
                nc.gpsimd.dma_start(out=t[:], in_=src)
            encTs[ti] = t

        for ec in range(EC):
            nc.sync.dma_start(out=We0[:, ec, :], in_=weT[:, ec, 0:P])
        load_enc_tile(0, split=True)
        for ec in range(EC):
            nc.sync.dma_start(out=We16[:, ec, :], in_=weT[:, ec, :])
        nc.sync.dma_start(out=hidT[:], in_=hid.rearrange("(hc p) b -> p hc b", p=P))
        nc.sync.dma_start(out=bT32[:], in_=bvec.rearrange("(dc p) -> p dc", p=P))
        nc.sync.dma_start(out=v32[:], in_=vvec.rearrange("(dc p) -> p dc", p=P))
        for hc in range(DC):
            nc.sync.dma_start(out=Wh16[:, hc, :], in_=whT[:, hc, :])
        load_enc_tile(1)
        load_enc_tile(2)

        # ---- hWh^T = W_h^T @ hidden^T + b : [d, batch] -----------------
        # The PE matmuls are slotted into the main PE stream after tile
        # 0's dc0-5 groups (by which time W_h has landed); the bias-adds
        # run on ACT (Identity + per-partition bias, psum -> SBUF) and
        # are emitted BEFORE any tanh so the ACT stream starts with them
        # and every tanh sees a properly-tracked hwhb dependency.
        hwhb = consts.tile([P, DC, BL], f32)

        def emit_hwh():
            pss = []
            for dc in range(DC):
                ps = psumH.tile([P, BL], f32, tag="psh")
                for hc in range(DC):
                    nc.tensor.matmul(
                        ps[:],
                        Wh16[:, hc, dc * P : (dc + 1) * P],
                        hidT[:, hc, :],
                        start=(hc == 0),
                        stop=(hc == DC - 1),
                    )
                pss.append(ps)
            for dc in range(DC):
                nc.scalar.add(
                    out=hwhb[:, dc, :], in_=pss[dc][:], add=bT32[:, dc : dc + 1]
                )

        # ---- main loop -------------------------------------------------
        # Tile 0 is special: its 6 leading psum groups are emitted before
        # the hwh block, their tanh/v-dot chains after it.
        scores = None
        HWH_AT = 6  # psE groups emitted before the hwh block (== psumE bufs)

        def emit_psE(ti, dc, encT):
            psE = psumE.tile([P, ST], f32)
            for ec in range(EC):
                stat = (
                    We0[:, ec, :]
                    if (ti == 0 and dc == 0)
                    else We16[:, ec, dc * P : (dc + 1) * P]
                )
                nc.tensor.matmul(
                    psE[:],
                    stat,
                    encT[:, ec, :],
                    start=(ec == 0),
                    stop=(ec == EC - 1),
                )
            return psE

        def emit_tail(b, dc, psE, th, acc, acc16):
            nc.scalar.activation(
                th[:, dc, :],
                psE[:],
                mybir.ActivationFunctionType.Tanh,
                bias=hwhb[:, dc, b : b + 1],
            )
            if dc == 0:
                nc.vector.tensor_scalar_mul(
                    out=acc[:], in0=th[:, 0, :], scalar1=v32[:, 0:1]
                )
            else:
                # last link writes fp16 so the ones-matmul streams at
                # 1 cycle/row; a single final rounding, not accumulated
                nc.vector.scalar_tensor_tensor(
                    out=(acc16[:] if dc == DC - 1 else acc[:]),
                    in0=th[:, dc, :],
                    scalar=v32[:, dc : dc + 1],
                    in1=acc[:],
                    op0=mybir.AluOpType.mult,
                    op1=mybir.AluOpType.add,
                )

        # Deferred partition-sum: ones^T @ acc16 replicates the 128-way
        # column sum across all psum partitions. Emitted two psum groups
        # into the NEXT tile so the PE never waits on the DVE chain.
        pending = None  # (b, st, acc16, scores)

        def flush_reduce():
            nonlocal pending
            if pending is None:
                return
            pb, pst, pacc16, pscores = pending
            pending = None
            psS = psumS.tile([P, ST], f32, tag="psS")
            nc.tensor.matmul(psS[:], ones16[:], pacc16[:], start=True, stop=True)
            nc.vector.tensor_copy(
                out=pscores[:, pst * ST : (pst + 1) * ST], in_=psS[:]
            )
            if pst == NST - 1:
                # ---- softmax on the replicated [128, S] rows ----------
                negmx = smallp.tile([P, 1], f32, tag="negmx")
                nc.vector.tensor_reduce(
                    out=negmx[:],
                    in_=pscores[:],
                    axis=mybir.AxisListType.X,
                    op=mybir.AluOpType.max,
                    negate=True,
                )
                probs = probp.tile([P, S], f32, tag="probs")
                ssum = smallp.tile([P, 1], f32, tag="ssum")
                nc.scalar.activation(
                    probs[:],
                    pscores[:],
                    mybir.ActivationFunctionType.Exp,
                    bias=negmx[:],
                    accum_out=ssum[:],
                )
                rec = smallp.tile([P, 1], f32, tag="rec")
                nc.vector.reciprocal(out=rec[:], in_=ssum[:])
                nc.vector.tensor_scalar_mul(out=probs[:], in0=probs[:], scalar1=rec[:])
                nc.sync.dma_start(out=out[pb, :], in_=probs[0:1, :])

        for ti, (b, st) in enumerate(tiles):
            if ti + 2 < len(tiles) and (ti + 2) not in encTs:
                load_enc_tile(ti + 2)
            encT = encTs.pop(ti)
            if st == 0:
                scores = scorep.tile([P, S], f32, tag="scores")
            th = thp.tile([P, DC, ST], f16, tag="th")
            acc = accp.tile([P, ST], f32, tag="acc")
            acc16 = acc16p.tile([P, ST], f16, tag="acc16")
            if ti == 0:
                pses = [emit_psE(0, dc, encT) for dc in range(HWH_AT)]
                emit_hwh()
                for dc in range(HWH_AT):
                    emit_tail(b, dc, pses[dc], th, acc, acc16)
                for dc in range(HWH_AT, DC):
                    psE = emit_psE(0, dc, encT)
                    emit_tail(b, dc, psE, th, acc, acc16)
            else:
                for dc in range(DC):
                    psE = emit_psE(ti, dc, encT)
                    if dc == 2:
                        flush_reduce()
                    emit_tail(b, dc, psE, th, acc, acc16)
            pending = (b, st, acc16, scores)
        flush_reduce()

    _split_multiwaits(nc)
    return nc


_NC = None


def _get_nc():
    global _NC
    if _NC is None:
        _NC = _build()
    return _NC


def make_in_maps(hidden, encoder_outputs, attn_w, attn_b, v):
    """Shard + marshal the full inputs into per-core device layouts."""
    hidden = np.asarray(hidden, dtype=np.float32)
    attn_w = np.asarray(attn_w, dtype=np.float32)
    attn_b = np.asarray(attn_b, dtype=np.float32)
    v = np.asarray(v, dtype=np.float32)
    enc16 = np.asarray(encoder_outputs, dtype=np.float16)  # [S, B, E]
    we16 = np.ascontiguousarray(attn_w[D:], dtype=np.float16)
    wh16 = np.ascontiguousarray(attn_w[:D], dtype=np.float16)
    hidT16 = np.ascontiguousarray(hidden.T, dtype=np.float16)  # [D, B]
    bT32 = np.ascontiguousarray(attn_b)
    v32 = np.ascontiguousarray(v)
    in_maps = []
    for c in range(NCORES):
        bsl = slice(c * BL, (c + 1) * BL)
        # per-batch 2D transposes keep the working set cache-sized
        enc_c = np.stack(
            [enc16[:, bb, :].T for bb in range(c * BL, (c + 1) * BL)]
        )  # [BL, E, S]
        in_maps.append(
            {
                "enc16": enc_c,
                "we16": we16,
                "wh16": wh16,
                "hidT16": np.ascontiguousarray(hidT16[:, bsl]),
                "bT32": bT32,
                "v32": v32,
            }
        )
    return in_maps


def kernel(hidden, encoder_outputs, attn_w, attn_b, v):
    nc = _get_nc()
    in_maps = make_in_maps(hidden, encoder_outputs, attn_w, attn_b, v)
    res = run_bass_kernel_spmd(nc, in_maps, core_ids=list(range(NCORES)))
    return np.concatenate(
        [res.results[c]["out"] for c in range(NCORES)], axis=0
    ).astype(np.float32)
